# revision 11
# baseline (speedup 1.0000x reference)
"""Multi-head attention (B=2, S=2048, H=1024, 16 heads) on 8 TRN2 NeuronCores.

Sharding: tensor-parallel over heads x data-parallel over batch.
core = b * 4 + g handles batch b and head-group g (4 heads, 256 channels).

Device-side dataflow (bf16 operands, fp32 PSUM accumulation):
  - Everything stays in "transposed space" so every matmul contracts over the
    partition dim with no on-device transposes:
      x_t    [H, S]      = hidden[b].T                      (host-transposed)
      qk_T   [512, S]    = (Wqk_g x_t)                      rows: q(4 heads), k(4 heads)
      v      [S, 256]    = x w_v.T  (natural layout; lhsT = x_t chunks)
      st     [128k, q]   = k_T_h^T-contracted scores (transposed scores)
      pt     = exp(st * scale + mask[k])                    (ACT, bias = per-partition mask)
      av     [128, q]    = v_aug^T pt ; rows 0:64 = unnormalized out.T,
                           rows 64:128 = Z[q] replicated (v_aug cols 64:128 == 1)
      attn_T [256, S]    = av[:64] * reciprocal(av[64:128])
      out_t  [H, S]      = Wo_g^T-contracted partial output (transposed)
  - Host sums the 4 group partials per batch, transposes back, and adds the
    exact bias corrections: b_out plus w_out @ b_v (the ones-augmented-V
    identity makes the v-bias a constant channel offset).

Scheduling: the softmax exp stream on the scalar engine (128 ACTs of
[128,1024], ~143us) is the pacing resource; the PE matmul stream (~140us)
is nearly co-paced.  So: a warmup matmul spinner brings the PE clock out of
its throttled state before real work, x arrives in token quarters
interleaved with the wqk chunks so the first attention window starts
~16us in, every other matmul (later qkv projections, out-projection) is
doled out in sub-group units between attention slots ordered by need-time,
each window's trailing AV matmuls + normalization spill into the next
window's first slots, and the last window takes a short-latency
normalization path so the tail stays small.
"""

import numpy as np

import concourse.tile as tile
from concourse import bacc, mybir
from concourse.bass_utils import run_bass_kernel_spmd

B, S, H = 2, 2048, 1024
NH, HD = 16, 64
NCORES = 8
NGROUP = 4              # head groups = cores per batch
HPG = NH // NGROUP      # 4 heads per group
DG = HPG * HD           # 256 channels per group
P = 128
SCALE = float(HD) ** -0.5

FP32 = mybir.dt.float32
BF16 = mybir.dt.bfloat16

S_TILES = S // P        # 16 key/token tiles
HC = H // P             # 8 contraction chunks over H
QKR = 2 * DG            # 512 q+k rows
QKC = QKR // P          # 4 chunks of qk rows
TQ = 512                # token quarter (qkv window / q window)
NQT = S // TQ           # 4

_NC_CACHE = None
LAST_RESULT = None      # BassKernelResults of the most recent run (for test.py)


def _body(tc, x_t, wqk_t, wv_t, wo_t, bqk, mask, out_t):
    nc = tc.nc
    with (
        tc.tile_pool(name="const", bufs=1) as const,
        tc.tile_pool(name="big", bufs=1) as big,
        tc.tile_pool(name="pt_pool", bufs=10) as pt_pool,
        tc.tile_pool(name="rz_pool", bufs=2) as rz_pool,
        tc.tile_pool(name="osb_pool", bufs=2) as osb_pool,
        tc.tile_pool(name="ps", bufs=2, space="PSUM") as ps,
        tc.tile_pool(name="avps", bufs=2, space="PSUM") as avps,
        tc.tile_pool(name="iops", bufs=2, space="PSUM") as iops,
    ):
        # ---------- PE warmup + ACT table preload ----------
        # The PE clock sits at half rate until ~3.4us of sustained activity;
        # the DMA-gated front would otherwise run every matmul cold.  Spin
        # dummy matmuls (nothing reads warm_ps) while the first inputs load.
        # A dummy exp also pulls the ACT table load off the critical path.
        warm_sb = const.tile([P, P], BF16, name="warm_sb")
        nc.vector.memset(warm_sb[:], 0.0)
        scr_sb = const.tile([P, 8], FP32, name="scr_sb")
        nc.scalar.activation(
            scr_sb[:], warm_sb[:, 0:8],
            mybir.ActivationFunctionType.Exp,
        )
        warm_ps = iops.tile([P, 512], FP32, name="warm_ps", tag="io")
        for i in range(56):
            nc.tensor.matmul(
                warm_ps[:, 0:P], lhsT=warm_sb[:], rhs=warm_sb[:],
                start=(i == 0), stop=(i == 55),
            )

        # ---------- input DMAs ----------
        # Descriptor generation is ~0.65us per dma_start on the issuing
        # engine; the scalar engine issues nothing (it must be free for the
        # exp stream).  The first matmuls need wqk chunks 0,2 (q+k of heads
        # 0,1) and x token-quarter 0, so those interleave across the sync
        # and gpsimd queues first; later quarters/weights follow in
        # first-use order.
        x_sb = big.tile([P, HC, S], BF16, name="x_sb")
        x_r = x_t.rearrange("(c p) s -> p c s", p=P)
        wqk_r = wqk_t.rearrange("(c p) r -> p c r", p=P)
        wqk_sb = const.tile([P, HC, QKR], BF16, name="wqk_sb")
        bqk_sb = const.tile([P, QKC], FP32, name="bqk_sb")
        mask_sb = const.tile([P, S_TILES], FP32, name="mask_sb")
        wv_sb = const.tile([P, HC, DG], BF16, name="wv_sb")
        wo_sb = const.tile([P, DG // P, H], BF16, name="wo_sb")

        def xdma(eng, tq, hcs):
            for hc in hcs:
                eng.dma_start(x_sb[:, hc, tq * TQ:(tq + 1) * TQ],
                              x_r[:, hc, tq * TQ:(tq + 1) * TQ])

        def wqkdma(eng, rc, hcl, hch):
            eng.dma_start(wqk_sb[:, hcl:hch, rc * P:(rc + 1) * P],
                          wqk_r[:, hcl:hch, rc * P:(rc + 1) * P])

        lo, hi = range(4), range(4, 8)
        # scalar queue is idle until the first exp (~13us): it takes half of
        # token-quarter 0 so the bootstrap unblocks fastest
        xdma(nc.scalar, 0, (2, 3, 6, 7))
        # sync queue: wqk chunk 0 (q heads 0,1) gates the very first matmul;
        # chunk 2 (k heads 0,1) is only needed by the second group, so it
        # rides behind the first x blocks.  bqk rides early (the first
        # psum evacuation adds it).
        wqkdma(nc.sync, 0, 0, 4)
        xdma(nc.sync, 0, (0,))
        nc.sync.dma_start(bqk_sb[:], bqk.rearrange("(c p) -> p c", p=P))
        xdma(nc.sync, 0, (1,))
        wqkdma(nc.sync, 2, 0, 4)
        nc.sync.dma_start(mask_sb[:], mask.rearrange("(c p) -> p c", p=P))
        xdma(nc.sync, 1, lo)
        wqkdma(nc.sync, 1, 0, 4)
        wqkdma(nc.sync, 3, 0, 4)
        xdma(nc.sync, 2, lo)
        xdma(nc.sync, 3, lo)
        # gpsimd queue
        wqkdma(nc.gpsimd, 0, 4, 8)
        xdma(nc.gpsimd, 0, (4, 5))
        wqkdma(nc.gpsimd, 2, 4, 8)
        nc.gpsimd.dma_start(wv_sb[:], wv_t.rearrange("(c p) r -> p c r", p=P))
        xdma(nc.gpsimd, 1, hi)
        wqkdma(nc.gpsimd, 1, 4, 8)
        wqkdma(nc.gpsimd, 3, 4, 8)
        xdma(nc.gpsimd, 2, hi)
        xdma(nc.gpsimd, 3, hi)
        nc.gpsimd.dma_start(wo_sb[:], wo_t.rearrange("(c p) r -> p c r", p=P))

        qk_sb = big.tile([P, QKC, S], BF16, name="qk_sb")
        # v_aug: per token tile / head: [v (64 cols) | ones (64 cols)]
        v_sb = big.tile([P, S_TILES, HPG, 2 * HD], BF16, name="v_sb")
        attn_sb = big.tile([P, DG // P, S], BF16, name="attn_sb")

        # ones half of v_aug in one strided memset
        nc.vector.memset(v_sb[:, :, :, HD:2 * HD], 1.0)

        # ---------- projection building blocks ----------
        def v_unit(tt):
            """v for one 128-token tile (8 matmuls, ~1us of PE)."""
            v_ps = iops.tile([P, 512], FP32, name="v_ps", tag="io")
            for hc in range(HC):
                nc.tensor.matmul(
                    v_ps[:, 0:DG],
                    lhsT=x_sb[:, hc, tt * P:(tt + 1) * P],
                    rhs=wv_sb[:, hc, :],
                    start=(hc == 0),
                    stop=(hc == HC - 1),
                )
            nc.vector.tensor_copy(
                v_sb[:, tt, :, 0:HD],
                v_ps[:, 0:DG].rearrange("p (h d) -> p h d", d=HD),
            )

        qk_state = {}

        def qk_part(rc, i, part):
            """Half of a qk projection group (4 of 8 contraction matmuls)."""
            if part == 0:
                qk_state[(rc, i)] = iops.tile(
                    [P, 512], FP32, name="qk_ps", tag="io")
            qk_ps = qk_state[(rc, i)]
            for hc in range(4 * part, 4 * part + 4):
                nc.tensor.matmul(
                    qk_ps[:],
                    lhsT=wqk_sb[:, hc, rc * P:(rc + 1) * P],
                    rhs=x_sb[:, hc, i * TQ:(i + 1) * TQ],
                    start=(hc == 0),
                    stop=(hc == HC - 1),
                )
            if part == 1:
                nc.vector.tensor_scalar_add(
                    qk_sb[:, rc, i * TQ:(i + 1) * TQ],
                    qk_ps[:],
                    bqk_sb[:, rc:rc + 1],
                )
                del qk_state[(rc, i)]

        def qk_group(rc, i):
            qk_part(rc, i, 0)
            qk_part(rc, i, 1)

        o_r = out_t.rearrange("(c p) s -> p c s", p=P)
        o_state = {}

        def out_proj_unit(q5, j):
            """One H-chunk (j) of the out-projection for q window q5.
            Both attn chunks must be complete in that window."""
            if j == 0:
                o_state[q5] = osb_pool.tile(
                    [P, H // P, 512], BF16, name=f"o_sb{q5}", tag="osb")
            o_sb = o_state[q5]
            qlo = q5 * 512
            o_ps = iops.tile([P, 512], FP32, name="o_ps", tag="io")
            for kc in range(DG // P):
                nc.tensor.matmul(
                    o_ps[:],
                    lhsT=wo_sb[:, kc, j * P:(j + 1) * P],
                    rhs=attn_sb[:, kc, qlo:qlo + 512],
                    start=(kc == 0),
                    stop=(kc == DG // P - 1),
                )
            nc.vector.tensor_copy(o_sb[:, j, :], o_ps[:])
            if j % 2 == 1:
                eng = nc.sync if (j // 2) % 2 == 0 else nc.gpsimd
                eng.dma_start(
                    o_r[:, j - 1:j + 1, qlo:qlo + 512], o_sb[:, j - 1:j + 1, :])

        # ---------- attention window ----------
        # Heads (2*qc, 2*qc+1) live at partition offsets 0/64 of qk chunk qc,
        # so their score matmuls land in disjoint row groups (tile_position
        # (0,0) / (64,0)) and execute concurrently.  Their 512-wide score
        # tiles sit side by side in one [128,1024] PSUM tile so a single
        # N=1024 exp covers both (mask bias depends only on the k-partition).
        def window(qc, q5, bg, avlag=4, last=False):
            """Attention for head pair qc, q window q5; bg maps kt -> list of
            background closures emitted after that kt's attention slots.

            The AV matmuls trail the score/exp stream by avlag slots so the
            first AV (which must wait for the previous window's av PSUM
            tiles to be released by its normalization) never head-of-line
            blocks the next scores on the PE queue.  Trailing AVs and the
            normalization are returned as closures for the caller to emit
            inside the next window (or inline for the last one)."""
            qlo = q5 * 512
            av0 = avps.tile([P, 512], FP32, name="av0", tag="av")
            av1 = avps.tile([P, 512], FP32, name="av1", tag="av")
            pts = {}

            def av_mms(kts):
                for kt in kts:
                    for half, av in ((0, av0), (1, av1)):
                        nc.tensor.matmul(
                            av[:],
                            lhsT=v_sb[:, kt, 2 * qc + half, :],
                            rhs=pts[kt][:, half * 512:(half + 1) * 512],
                            start=(kt == 0),
                            stop=(kt == S_TILES - 1),
                        )
                    del pts[kt]

            for kt in range(S_TILES):
                st = ps.tile([P, 1024], FP32, name="st", tag="mm")
                for half in range(2):
                    off = half * HD
                    nc.tensor.matmul(
                        st[:, half * 512:(half + 1) * 512],
                        lhsT=qk_sb[off:off + HD, 2 + qc,
                                   kt * P:(kt + 1) * P],
                        rhs=qk_sb[off:off + HD, qc, qlo:qlo + 512],
                        start=True,
                        stop=True,
                    )
                pt = pt_pool.tile([P, 1024], BF16, name="pt", tag="pt")
                nc.scalar.activation(
                    pt[:], st[:],
                    mybir.ActivationFunctionType.Exp,
                    bias=mask_sb[:, kt:kt + 1],
                    scale=SCALE,
                )
                pts[kt] = pt
                if kt >= avlag:
                    av_mms([kt - avlag])
                for work in bg.get(kt, ()):
                    work()

            def norm():
                # Evacuate both av halves to SBUF (releasing the av PSUM
                # tiles after two DVE ops each), then reciprocal+multiply
                # run all-SBUF at 2x DVE rate.
                zcs, ocs = [], []
                for half, av in ((0, av0), (1, av1)):
                    zc = rz_pool.tile([HD, 512], FP32, name="zc", tag="zc",
                                      bufs=2)
                    nc.vector.tensor_copy(zc[:], av[HD:2 * HD, :])
                    oc = rz_pool.tile([HD, 512], FP32, name="oc", tag="oc",
                                      bufs=2)
                    nc.vector.tensor_copy(oc[:], av[0:HD, :])
                    zcs.append(zc)
                    ocs.append(oc)
                for half in range(2):
                    off = half * HD
                    rz = rz_pool.tile([HD, 512], FP32, name="rz", tag="rz",
                                      bufs=2)
                    nc.vector.reciprocal_approx_fast(rz[:], zcs[half][:])
                    nc.vector.tensor_mul(
                        attn_sb[off:off + HD, qc, qlo:qlo + 512],
                        ocs[half][:],
                        rz[:],
                    )

            def norm_fast():
                # Short-latency variant for the last window: zc copies run
                # on scalar (idle after its last exp) and vector in
                # parallel; multiplies read av directly from PSUM.
                zcs = []
                for half, av, eng in ((0, av0, nc.scalar), (1, av1, None)):
                    zc = rz_pool.tile([HD, 512], FP32, name="zcf", tag="zc",
                                      bufs=2)
                    if eng is nc.scalar:
                        nc.scalar.activation(
                            zc[:], av[HD:2 * HD, :],
                            mybir.ActivationFunctionType.Copy,
                        )
                    else:
                        nc.vector.tensor_copy(zc[:], av[HD:2 * HD, :])
                    zcs.append(zc)
                for half, av in ((0, av0), (1, av1)):
                    off = half * HD
                    rz = rz_pool.tile([HD, 512], FP32, name="rzf", tag="rz",
                                      bufs=2)
                    nc.vector.reciprocal_approx_fast(rz[:], zcs[half][:])
                    nc.vector.tensor_mul(
                        attn_sb[off:off + HD, qc, qlo:qlo + 512],
                        av[0:HD, :],
                        rz[:],
                    )

            rest = [kt for kt in range(S_TILES - avlag, S_TILES)]
            if last:
                av_mms(rest)
                norm_fast()
                return []
            return [
                lambda: av_mms(rest[0:2]),
                lambda: av_mms(rest[2:4]),
                norm,
            ]

        # ---------- schedule ----------
        # Bootstrap: q h01 window 0 + k h01 tiles 0-3 (gated only on x
        # token-quarter 0 + wqk chunks 0,2); everything else trickles in as
        # background units ordered by first-use time.
        qk_group(0, 0)
        qk_group(2, 0)

        V = v_unit

        def G(rc, i, part):
            return lambda: qk_part(rc, i, part)

        def op(q5):
            return [lambda j=j: out_proj_unit(q5, j) for j in range(H // P)]

        # Background plans per window, keyed by kt slot.  Window (0,0)
        # carries the x-gated rest of the k/v projection just-in-time
        # (k tiles 4*i arrive via qk chunk-2 parts ahead of first use; AV
        # trails the exp stream by avlag slots so v units also fit).
        bgs = {
            (0, 0): {0: [lambda: V(0)], 1: [lambda: V(1)],
                     2: [G(2, 1, 0)], 3: [G(2, 1, 1), lambda: V(2)],
                     4: [lambda: V(3)], 5: [lambda: V(4)],
                     6: [G(2, 2, 0)], 7: [G(2, 2, 1), lambda: V(5)],
                     8: [lambda: V(6)], 9: [lambda: V(7)],
                     10: [G(2, 3, 0)], 11: [G(2, 3, 1), lambda: V(8)],
                     12: [lambda: V(9), lambda: V(10)],
                     13: [lambda: V(11), lambda: V(12)],
                     14: [lambda: V(13), G(0, 1, 0)],
                     15: [lambda: V(14), lambda: V(15), G(0, 1, 1)]},
            (0, 1): {6: [G(0, 2, 0)], 7: [G(0, 2, 1)]},
            (0, 2): {4: [G(0, 3, 0)], 5: [G(0, 3, 1)],
                     7: [G(1, 0, 0)], 8: [G(1, 0, 1)],
                     10: [G(3, 0, 0)], 11: [G(3, 0, 1)]},
            (0, 3): {4: [G(3, 1, 0)], 5: [G(3, 1, 1)],
                     7: [G(3, 2, 0)], 8: [G(3, 2, 1)],
                     10: [G(3, 3, 0)], 11: [G(3, 3, 1)],
                     13: [G(1, 1, 0)], 14: [G(1, 1, 1)]},
            (1, 0): {4: [G(1, 2, 0)], 5: [G(1, 2, 1)],
                     8: [G(1, 3, 0)], 9: [G(1, 3, 1)]},
        }
        # out-projection of q window q5 spread through window (1, q5+1),
        # starting late enough that the previous window's normalization
        # (deferred into this window's slots 0-2) has completed.
        for q5 in range(3):
            units = op(q5)
            d = bgs.setdefault((1, q5 + 1), {})
            for j in range(8):
                d.setdefault(5 + j, []).append(units[j])

        finishers = []
        for qc in range(2):
            for q5 in range(NQT):
                bg = dict(bgs.get((qc, q5), {}))
                for slot, work in enumerate(finishers):
                    bg.setdefault(slot, []).insert(0, work)
                last = (qc, q5) == (1, NQT - 1)
                finishers = window(qc, q5, bg,
                                   avlag=2 if last else 4, last=last)

        # tail: out-projection of the last q window
        for u in op(3):
            u()


def _build():
    nc = bacc.Bacc(
        "TRN2",
        target_bir_lowering=False,
        debug=False,
        enable_asserts=True,
        num_devices=NCORES,
    )
    x_t = nc.dram_tensor("x_t", [H, S], BF16, kind="ExternalInput").ap()
    wqk_t = nc.dram_tensor("wqk_t", [H, QKR], BF16, kind="ExternalInput").ap()
    wv_t = nc.dram_tensor("wv_t", [H, DG], BF16, kind="ExternalInput").ap()
    wo_t = nc.dram_tensor("wo_t", [DG, H], BF16, kind="ExternalInput").ap()
    bqk = nc.dram_tensor("bqk", [QKR], FP32, kind="ExternalInput").ap()
    mask = nc.dram_tensor("mask", [S], FP32, kind="ExternalInput").ap()
    out_t = nc.dram_tensor("out_t", [H, S], BF16, kind="ExternalOutput").ap()

    with tile.TileContext(nc) as tc:
        _body(tc, x_t, wqk_t, wv_t, wo_t, bqk, mask, out_t)
    nc.compile()
    return nc


def _get_nc():
    global _NC_CACHE
    if _NC_CACHE is None:
        _NC_CACHE = _build()
    return _NC_CACHE


def make_in_maps(hidden_states, attention_mask, w_qkv, b_qkv, w_out):
    import ml_dtypes

    bf16 = ml_dtypes.bfloat16
    in_maps = []
    for core in range(NCORES):
        b, g = divmod(core, NGROUP)
        wq = w_qkv[0 * H + g * DG:0 * H + (g + 1) * DG]
        wk = w_qkv[1 * H + g * DG:1 * H + (g + 1) * DG]
        wv = w_qkv[2 * H + g * DG:2 * H + (g + 1) * DG]
        in_maps.append({
            "x_t": np.ascontiguousarray(hidden_states[b].T).astype(bf16),
            "wqk_t": np.ascontiguousarray(
                np.concatenate([wq, wk], 0).T).astype(bf16),
            "wv_t": np.ascontiguousarray(wv.T).astype(bf16),
            "wo_t": np.ascontiguousarray(
                w_out[:, g * DG:(g + 1) * DG].T).astype(bf16),
            "bqk": np.ascontiguousarray(
                np.concatenate([b_qkv[g * DG:(g + 1) * DG],
                                b_qkv[H + g * DG:H + (g + 1) * DG]])),
            "mask": np.ascontiguousarray(attention_mask[b]),
        })
    return in_maps


def kernel(hidden_states, attention_mask, w_qkv, b_qkv, w_out, b_out):
    global LAST_RESULT
    hidden_states = np.asarray(hidden_states, dtype=np.float32)
    attention_mask = np.asarray(attention_mask, dtype=np.float32)
    w_qkv = np.asarray(w_qkv, dtype=np.float32)
    b_qkv = np.asarray(b_qkv, dtype=np.float32)
    w_out = np.asarray(w_out, dtype=np.float32)
    b_out = np.asarray(b_out, dtype=np.float32)

    nc = _get_nc()
    in_maps = make_in_maps(hidden_states, attention_mask, w_qkv, b_qkv, w_out)

    import os
    trace = bool(int(os.environ.get("KERNEL_TRACE", "0")))
    res = run_bass_kernel_spmd(
        nc, in_maps, core_ids=list(range(NCORES)), trace=trace,
    )
    LAST_RESULT = res

    out = np.zeros((B, S, H), np.float32)
    vbias = w_out @ b_qkv[2 * H:]          # exact v-bias correction
    for b in range(B):
        acc = res.results[b * NGROUP + 0]["out_t"].astype(np.float32)
        for g in range(1, NGROUP):
            acc = acc + res.results[b * NGROUP + g]["out_t"].astype(np.float32)
        out[b] = acc.T + b_out + vbias
    return out


# revision 14
# speedup vs baseline: 1.0329x; 1.0329x over previous
"""Multi-head attention (B=2, S=2048, H=1024, 16 heads) on 8 TRN2 NeuronCores.

Sharding: tensor-parallel over heads x data-parallel over batch.
core = b * 4 + g handles batch b and head-group g (4 heads, 256 channels).

Device-side dataflow (bf16 operands, fp32 PSUM accumulation):
  - Everything stays in "transposed space" so every matmul contracts over the
    partition dim with no on-device transposes:
      x_t    [H, S]      = hidden[b].T                      (host-transposed)
      qk_T   [512, S]    = (Wqk_g x_t)                      rows: q(4 heads), k(4 heads)
      v      [S, 256]    = x w_v.T  (natural layout; lhsT = x_t chunks)
      st     [128k, q]   = k_T_h^T-contracted scores (transposed scores)
      pt     = exp(st * scale + mask[k])                    (ACT, bias = per-partition mask)
      av     [128, q]    = v_aug^T pt ; rows 0:64 = unnormalized out.T,
                           rows 64:128 = Z[q] replicated (v_aug cols 64:128 == 1)
      attn_T [256, S]    = av[:64] * reciprocal(av[64:128])
      out_t  [H, S]      = Wo_g^T-contracted partial output (transposed)
  - Host sums the 4 group partials per batch, transposes back, and adds the
    exact bias corrections: b_out plus w_out @ b_v (the ones-augmented-V
    identity makes the v-bias a constant channel offset).

Scheduling: the softmax exp stream on the scalar engine (128 ACTs of
[128,1024], ~143us) is the pacing resource; the PE matmul stream (~140us)
is nearly co-paced.  So: a warmup matmul spinner brings the PE clock out of
its throttled state before real work, x arrives in token quarters
interleaved with the wqk chunks so the first attention window starts
~16us in, every other matmul (later qkv projections, out-projection) is
doled out in sub-group units between attention slots ordered by need-time,
each window's trailing AV matmuls + normalization spill into the next
window's first slots, and the last window takes a short-latency
normalization path so the tail stays small.
"""

import numpy as np

import concourse.tile as tile
from concourse import bacc, mybir
from concourse.bass_utils import run_bass_kernel_spmd

B, S, H = 2, 2048, 1024
NH, HD = 16, 64
NCORES = 8
NGROUP = 4              # head groups = cores per batch
HPG = NH // NGROUP      # 4 heads per group
DG = HPG * HD           # 256 channels per group
P = 128
SCALE = float(HD) ** -0.5

FP32 = mybir.dt.float32
BF16 = mybir.dt.bfloat16

S_TILES = S // P        # 16 key/token tiles
HC = H // P             # 8 contraction chunks over H
QKR = 2 * DG            # 512 q+k rows
QKC = QKR // P          # 4 chunks of qk rows
TQ = 512                # token quarter (qkv window / q window)
NQT = S // TQ           # 4

_NC_CACHE = None
LAST_RESULT = None      # BassKernelResults of the most recent run (for test.py)


def _body(tc, x_t, wqk_t, wv_t, wo_t, bqk, mask, out_t):
    nc = tc.nc
    with (
        tc.tile_pool(name="const", bufs=1) as const,
        tc.tile_pool(name="big", bufs=1) as big,
        tc.tile_pool(name="pt_pool", bufs=10) as pt_pool,
        tc.tile_pool(name="rz_pool", bufs=2) as rz_pool,
        tc.tile_pool(name="osb_pool", bufs=2) as osb_pool,
        tc.tile_pool(name="ps", bufs=2, space="PSUM") as ps,
        tc.tile_pool(name="avps", bufs=2, space="PSUM") as avps,
        tc.tile_pool(name="iops", bufs=2, space="PSUM") as iops,
    ):
        # ---------- PE warmup + ACT table preload ----------
        # The PE clock sits at half rate until ~3.4us of sustained activity;
        # the DMA-gated front would otherwise run every matmul cold.  Spin
        # dummy matmuls (nothing reads warm_ps) while the first inputs load.
        # A dummy exp also pulls the ACT table load off the critical path.
        warm_sb = const.tile([P, P], BF16, name="warm_sb")
        nc.vector.memset(warm_sb[:], 0.0)
        scr_sb = const.tile([P, 8], FP32, name="scr_sb")
        nc.scalar.activation(
            scr_sb[:], warm_sb[:, 0:8],
            mybir.ActivationFunctionType.Exp,
        )
        warm_ps = iops.tile([P, 512], FP32, name="warm_ps", tag="io")
        for i in range(56):
            nc.tensor.matmul(
                warm_ps[:, 0:P], lhsT=warm_sb[:], rhs=warm_sb[:],
                start=(i == 0), stop=(i == 55),
            )

        # ---------- input DMAs ----------
        # Descriptor generation is ~0.65us per dma_start on the issuing
        # engine; the scalar engine issues nothing (it must be free for the
        # exp stream).  The first matmuls need wqk chunks 0,2 (q+k of heads
        # 0,1) and x token-quarter 0, so those interleave across the sync
        # and gpsimd queues first; later quarters/weights follow in
        # first-use order.
        x_sb = big.tile([P, HC, S], BF16, name="x_sb")
        x_r = x_t.rearrange("(c p) s -> p c s", p=P)
        wqk_r = wqk_t.rearrange("(c p) r -> p c r", p=P)
        wqk_sb = const.tile([P, HC, QKR], BF16, name="wqk_sb")
        bqk_sb = const.tile([P, QKC], FP32, name="bqk_sb")
        mask_sb = const.tile([P, S_TILES], FP32, name="mask_sb")
        wv_sb = const.tile([P, HC, DG], BF16, name="wv_sb")
        wo_sb = const.tile([P, DG // P, H], BF16, name="wo_sb")

        def xdma(eng, tq, hcs):
            for hc in hcs:
                eng.dma_start(x_sb[:, hc, tq * TQ:(tq + 1) * TQ],
                              x_r[:, hc, tq * TQ:(tq + 1) * TQ])

        def wqkdma(eng, rc, hcl, hch):
            eng.dma_start(wqk_sb[:, hcl:hch, rc * P:(rc + 1) * P],
                          wqk_r[:, hcl:hch, rc * P:(rc + 1) * P])

        lo, hi = range(4), range(4, 8)
        # scalar queue is idle until the first exp (~13us): it takes half of
        # token-quarter 0 so the bootstrap unblocks fastest
        xdma(nc.scalar, 0, (2, 3, 6, 7))
        # sync queue: wqk chunk 0 (q heads 0,1) gates the very first matmul;
        # chunk 2 (k heads 0,1) is only needed by the second group, so it
        # rides behind the first x blocks.  bqk rides early (the first
        # psum evacuation adds it).
        wqkdma(nc.sync, 0, 0, 4)
        xdma(nc.sync, 0, (0,))
        nc.sync.dma_start(bqk_sb[:], bqk.rearrange("(c p) -> p c", p=P))
        xdma(nc.sync, 0, (1,))
        wqkdma(nc.sync, 2, 0, 4)
        nc.sync.dma_start(mask_sb[:], mask.rearrange("(c p) -> p c", p=P))
        xdma(nc.sync, 1, lo)
        wqkdma(nc.sync, 1, 0, 4)
        wqkdma(nc.sync, 3, 0, 4)
        xdma(nc.sync, 2, lo)
        xdma(nc.sync, 3, lo)
        # gpsimd queue
        wqkdma(nc.gpsimd, 0, 4, 8)
        xdma(nc.gpsimd, 0, (4, 5))
        wqkdma(nc.gpsimd, 2, 4, 8)
        nc.gpsimd.dma_start(wv_sb[:], wv_t.rearrange("(c p) r -> p c r", p=P))
        xdma(nc.gpsimd, 1, hi)
        wqkdma(nc.gpsimd, 1, 4, 8)
        wqkdma(nc.gpsimd, 3, 4, 8)
        xdma(nc.gpsimd, 2, hi)
        xdma(nc.gpsimd, 3, hi)
        nc.gpsimd.dma_start(wo_sb[:], wo_t.rearrange("(c p) r -> p c r", p=P))

        qk_sb = big.tile([P, QKC, S], BF16, name="qk_sb")
        # v_aug: per token tile / head: [v (64 cols) | ones (64 cols)]
        v_sb = big.tile([P, S_TILES, HPG, 2 * HD], BF16, name="v_sb")
        attn_sb = big.tile([P, DG // P, S], BF16, name="attn_sb")

        # ones half of v_aug in one strided memset
        nc.vector.memset(v_sb[:, :, :, HD:2 * HD], 1.0)

        # ---------- projection building blocks ----------
        def v_unit(tt):
            """v for one 128-token tile (8 matmuls, ~1us of PE)."""
            v_ps = iops.tile([P, 512], FP32, name="v_ps", tag="io")
            for hc in range(HC):
                nc.tensor.matmul(
                    v_ps[:, 0:DG],
                    lhsT=x_sb[:, hc, tt * P:(tt + 1) * P],
                    rhs=wv_sb[:, hc, :],
                    start=(hc == 0),
                    stop=(hc == HC - 1),
                )
            nc.vector.tensor_copy(
                v_sb[:, tt, :, 0:HD],
                v_ps[:, 0:DG].rearrange("p (h d) -> p h d", d=HD),
            )

        qk_state = {}

        def qk_part(rc, i, part, order=None):
            """Half of a qk projection group (4 of 8 contraction matmuls).
            `order` is the hc sequence split across the two parts — the
            bootstrap groups follow expected x-chunk DMA arrival order."""
            if order is None:
                order = range(HC)
            order = list(order)
            if part == 0:
                qk_state[(rc, i)] = iops.tile(
                    [P, 512], FP32, name="qk_ps", tag="io")
            qk_ps = qk_state[(rc, i)]
            for j in range(4 * part, 4 * part + 4):
                hc = order[j]
                nc.tensor.matmul(
                    qk_ps[:],
                    lhsT=wqk_sb[:, hc, rc * P:(rc + 1) * P],
                    rhs=x_sb[:, hc, i * TQ:(i + 1) * TQ],
                    start=(j == 0),
                    stop=(j == HC - 1),
                )
            if part == 1:
                nc.vector.tensor_scalar_add(
                    qk_sb[:, rc, i * TQ:(i + 1) * TQ],
                    qk_ps[:],
                    bqk_sb[:, rc:rc + 1],
                )
                del qk_state[(rc, i)]

        def qk_group(rc, i, order=None):
            qk_part(rc, i, 0, order)
            qk_part(rc, i, 1, order)

        o_r = out_t.rearrange("(c p) s -> p c s", p=P)
        o_state = {}

        def out_proj_unit(q5, j):
            """One H-chunk (j) of the out-projection for q window q5.
            Both attn chunks must be complete in that window."""
            if j == 0:
                o_state[q5] = osb_pool.tile(
                    [P, H // P, 512], BF16, name=f"o_sb{q5}", tag="osb")
            o_sb = o_state[q5]
            qlo = q5 * 512
            o_ps = iops.tile([P, 512], FP32, name="o_ps", tag="io")
            for kc in range(DG // P):
                nc.tensor.matmul(
                    o_ps[:],
                    lhsT=wo_sb[:, kc, j * P:(j + 1) * P],
                    rhs=attn_sb[:, kc, qlo:qlo + 512],
                    start=(kc == 0),
                    stop=(kc == DG // P - 1),
                )
            if q5 == NQT - 1 and j % 2 == 0:
                # tail path: the exp stream is over, so the scalar engine
                # helps with PSUM evacuation and the output DMA goes per-j
                # on alternating queues to shorten the final drain
                nc.scalar.activation(
                    o_sb[:, j, :], o_ps[:],
                    mybir.ActivationFunctionType.Copy,
                )
            else:
                nc.vector.tensor_copy(o_sb[:, j, :], o_ps[:])
            if q5 == NQT - 1:
                eng = nc.sync if j % 2 == 0 else nc.gpsimd
                eng.dma_start(o_r[:, j, qlo:qlo + 512], o_sb[:, j, :])
            elif j % 2 == 1:
                eng = nc.sync if (j // 2) % 2 == 0 else nc.gpsimd
                eng.dma_start(
                    o_r[:, j - 1:j + 1, qlo:qlo + 512], o_sb[:, j - 1:j + 1, :])

        # ---------- attention window ----------
        # Heads (2*qc, 2*qc+1) live at partition offsets 0/64 of qk chunk qc,
        # so their score matmuls land in disjoint row groups (tile_position
        # (0,0) / (64,0)) and execute concurrently.  Their 512-wide score
        # tiles sit side by side in one [128,1024] PSUM tile so a single
        # N=1024 exp covers both (mask bias depends only on the k-partition).
        def window(qc, q5, bg, avlag=4, last=False):
            """Attention for head pair qc, q window q5; bg maps kt -> list of
            background closures emitted after that kt's attention slots.

            The AV matmuls trail the score/exp stream by avlag slots so the
            first AV (which must wait for the previous window's av PSUM
            tiles to be released by its normalization) never head-of-line
            blocks the next scores on the PE queue.  Trailing AVs and the
            normalization are returned as closures for the caller to emit
            inside the next window (or inline for the last one)."""
            qlo = q5 * 512
            av0 = avps.tile([P, 512], FP32, name="av0", tag="av")
            av1 = avps.tile([P, 512], FP32, name="av1", tag="av")
            pts = {}

            def av_mms(kts):
                for kt in kts:
                    for half, av in ((0, av0), (1, av1)):
                        nc.tensor.matmul(
                            av[:],
                            lhsT=v_sb[:, kt, 2 * qc + half, :],
                            rhs=pts[kt][:, half * 512:(half + 1) * 512],
                            start=(kt == 0),
                            stop=(kt == S_TILES - 1),
                        )
                    del pts[kt]

            for kt in range(S_TILES):
                st = ps.tile([P, 1024], FP32, name="st", tag="mm")
                for half in range(2):
                    off = half * HD
                    nc.tensor.matmul(
                        st[:, half * 512:(half + 1) * 512],
                        lhsT=qk_sb[off:off + HD, 2 + qc,
                                   kt * P:(kt + 1) * P],
                        rhs=qk_sb[off:off + HD, qc, qlo:qlo + 512],
                        start=True,
                        stop=True,
                    )
                pt = pt_pool.tile([P, 1024], BF16, name="pt", tag="pt")
                nc.scalar.activation(
                    pt[:], st[:],
                    mybir.ActivationFunctionType.Exp,
                    bias=mask_sb[:, kt:kt + 1],
                    scale=SCALE,
                )
                pts[kt] = pt
                if kt >= avlag:
                    av_mms([kt - avlag])
                for work in bg.get(kt, ()):
                    work()

            def norm():
                # Evacuate both av halves to SBUF (releasing the av PSUM
                # tiles after two DVE ops each), then reciprocal+multiply
                # run all-SBUF at 2x DVE rate.
                zcs, ocs = [], []
                for half, av in ((0, av0), (1, av1)):
                    zc = rz_pool.tile([HD, 512], FP32, name="zc", tag="zc",
                                      bufs=2)
                    nc.vector.tensor_copy(zc[:], av[HD:2 * HD, :])
                    oc = rz_pool.tile([HD, 512], FP32, name="oc", tag="oc",
                                      bufs=2)
                    nc.vector.tensor_copy(oc[:], av[0:HD, :])
                    zcs.append(zc)
                    ocs.append(oc)
                for half in range(2):
                    off = half * HD
                    rz = rz_pool.tile([HD, 512], FP32, name="rz", tag="rz",
                                      bufs=2)
                    nc.vector.reciprocal_approx_fast(rz[:], zcs[half][:])
                    nc.vector.tensor_mul(
                        attn_sb[off:off + HD, qc, qlo:qlo + 512],
                        ocs[half][:],
                        rz[:],
                    )

            def norm_fast():
                # Short-latency variant for the last window: zc copies run
                # on scalar (idle after its last exp) and vector in
                # parallel; multiplies read av directly from PSUM.
                zcs = []
                for half, av, eng in ((0, av0, nc.scalar), (1, av1, None)):
                    zc = rz_pool.tile([HD, 512], FP32, name="zcf", tag="zc",
                                      bufs=2)
                    if eng is nc.scalar:
                        nc.scalar.activation(
                            zc[:], av[HD:2 * HD, :],
                            mybir.ActivationFunctionType.Copy,
                        )
                    else:
                        nc.vector.tensor_copy(zc[:], av[HD:2 * HD, :])
                    zcs.append(zc)
                for half, av in ((0, av0), (1, av1)):
                    off = half * HD
                    rz = rz_pool.tile([HD, 512], FP32, name="rzf", tag="rz",
                                      bufs=2)
                    nc.vector.reciprocal_approx_fast(rz[:], zcs[half][:])
                    nc.vector.tensor_mul(
                        attn_sb[off:off + HD, qc, qlo:qlo + 512],
                        av[0:HD, :],
                        rz[:],
                    )

            rest = [kt for kt in range(S_TILES - avlag, S_TILES)]
            if last:
                av_mms(rest)
                norm_fast()
                return []
            return [
                lambda: av_mms(rest[0:2]),
                lambda: av_mms(rest[2:4]),
                norm,
            ]

        # ---------- schedule ----------
        # Bootstrap: q h01 window 0 + k h01 tiles 0-3 (gated only on x
        # token-quarter 0 + wqk chunks 0,2); everything else trickles in as
        # background units ordered by first-use time.  The contraction order
        # follows expected x-chunk arrival (scalar queue lands hc 2,3,6,7
        # first) so the matmul chain starts as soon as the first chunks land.
        BOOT_ORDER = (2, 3, 6, 7, 4, 5, 0, 1)
        qk_group(0, 0, BOOT_ORDER)
        qk_group(2, 0, BOOT_ORDER)

        V = v_unit

        def G(rc, i, part):
            return lambda: qk_part(rc, i, part)

        def op(q5):
            return [lambda j=j: out_proj_unit(q5, j) for j in range(H // P)]

        # Background plans per window, keyed by kt slot.  Window (0,0)
        # carries the x-gated rest of the k/v projection just-in-time
        # (k tiles 4*i arrive via qk chunk-2 parts ahead of first use; AV
        # trails the exp stream by avlag slots so v units also fit).
        bgs = {
            (0, 0): {0: [lambda: V(0)], 1: [lambda: V(1)],
                     2: [G(2, 1, 0)], 3: [G(2, 1, 1), lambda: V(2)],
                     4: [lambda: V(3)], 5: [lambda: V(4)],
                     6: [G(2, 2, 0)], 7: [G(2, 2, 1), lambda: V(5)],
                     8: [lambda: V(6)], 9: [lambda: V(7)],
                     10: [G(2, 3, 0)], 11: [G(2, 3, 1), lambda: V(8)],
                     12: [lambda: V(9), lambda: V(10)],
                     13: [lambda: V(11), lambda: V(12)],
                     14: [lambda: V(13), G(0, 1, 0)],
                     15: [lambda: V(14), lambda: V(15), G(0, 1, 1)]},
            (0, 1): {6: [G(0, 2, 0)], 7: [G(0, 2, 1)]},
            (0, 2): {4: [G(0, 3, 0)], 5: [G(0, 3, 1)],
                     7: [G(1, 0, 0)], 8: [G(1, 0, 1)],
                     10: [G(3, 0, 0)], 11: [G(3, 0, 1)]},
            (0, 3): {4: [G(3, 1, 0)], 5: [G(3, 1, 1)],
                     7: [G(3, 2, 0)], 8: [G(3, 2, 1)],
                     10: [G(3, 3, 0)], 11: [G(3, 3, 1)],
                     13: [G(1, 1, 0)], 14: [G(1, 1, 1)]},
            (1, 0): {4: [G(1, 2, 0)], 5: [G(1, 2, 1)],
                     8: [G(1, 3, 0)], 9: [G(1, 3, 1)]},
        }
        # out-projection of q window q5 spread through window (1, q5+1),
        # starting late enough that the previous window's normalization
        # (deferred into this window's slots 0-2) has completed.
        for q5 in range(3):
            units = op(q5)
            d = bgs.setdefault((1, q5 + 1), {})
            for j in range(8):
                d.setdefault(5 + j, []).append(units[j])

        finishers = []
        for qc in range(2):
            for q5 in range(NQT):
                bg = dict(bgs.get((qc, q5), {}))
                for slot, work in enumerate(finishers):
                    bg.setdefault(slot, []).insert(0, work)
                last = (qc, q5) == (1, NQT - 1)
                finishers = window(qc, q5, bg,
                                   avlag=2 if last else 4, last=last)

        # tail: out-projection of the last q window
        for u in op(3):
            u()


def _build():
    nc = bacc.Bacc(
        "TRN2",
        target_bir_lowering=False,
        debug=False,
        enable_asserts=True,
        num_devices=NCORES,
    )
    x_t = nc.dram_tensor("x_t", [H, S], BF16, kind="ExternalInput").ap()
    wqk_t = nc.dram_tensor("wqk_t", [H, QKR], BF16, kind="ExternalInput").ap()
    wv_t = nc.dram_tensor("wv_t", [H, DG], BF16, kind="ExternalInput").ap()
    wo_t = nc.dram_tensor("wo_t", [DG, H], BF16, kind="ExternalInput").ap()
    bqk = nc.dram_tensor("bqk", [QKR], FP32, kind="ExternalInput").ap()
    mask = nc.dram_tensor("mask", [S], FP32, kind="ExternalInput").ap()
    out_t = nc.dram_tensor("out_t", [H, S], BF16, kind="ExternalOutput").ap()

    with tile.TileContext(nc) as tc:
        _body(tc, x_t, wqk_t, wv_t, wo_t, bqk, mask, out_t)
    nc.compile()
    return nc


def _get_nc():
    global _NC_CACHE
    if _NC_CACHE is None:
        _NC_CACHE = _build()
    return _NC_CACHE


def make_in_maps(hidden_states, attention_mask, w_qkv, b_qkv, w_out):
    import ml_dtypes

    bf16 = ml_dtypes.bfloat16
    in_maps = []
    for core in range(NCORES):
        b, g = divmod(core, NGROUP)
        wq = w_qkv[0 * H + g * DG:0 * H + (g + 1) * DG]
        wk = w_qkv[1 * H + g * DG:1 * H + (g + 1) * DG]
        wv = w_qkv[2 * H + g * DG:2 * H + (g + 1) * DG]
        in_maps.append({
            "x_t": np.ascontiguousarray(hidden_states[b].T).astype(bf16),
            "wqk_t": np.ascontiguousarray(
                np.concatenate([wq, wk], 0).T).astype(bf16),
            "wv_t": np.ascontiguousarray(wv.T).astype(bf16),
            "wo_t": np.ascontiguousarray(
                w_out[:, g * DG:(g + 1) * DG].T).astype(bf16),
            "bqk": np.ascontiguousarray(
                np.concatenate([b_qkv[g * DG:(g + 1) * DG],
                                b_qkv[H + g * DG:H + (g + 1) * DG]])),
            "mask": np.ascontiguousarray(attention_mask[b]),
        })
    return in_maps


def kernel(hidden_states, attention_mask, w_qkv, b_qkv, w_out, b_out):
    global LAST_RESULT
    hidden_states = np.asarray(hidden_states, dtype=np.float32)
    attention_mask = np.asarray(attention_mask, dtype=np.float32)
    w_qkv = np.asarray(w_qkv, dtype=np.float32)
    b_qkv = np.asarray(b_qkv, dtype=np.float32)
    w_out = np.asarray(w_out, dtype=np.float32)
    b_out = np.asarray(b_out, dtype=np.float32)

    nc = _get_nc()
    in_maps = make_in_maps(hidden_states, attention_mask, w_qkv, b_qkv, w_out)

    import os
    trace = bool(int(os.environ.get("KERNEL_TRACE", "0")))
    res = run_bass_kernel_spmd(
        nc, in_maps, core_ids=list(range(NCORES)), trace=trace,
    )
    LAST_RESULT = res

    out = np.zeros((B, S, H), np.float32)
    vbias = w_out @ b_qkv[2 * H:]          # exact v-bias correction
    for b in range(B):
        acc = res.results[b * NGROUP + 0]["out_t"].astype(np.float32)
        for g in range(1, NGROUP):
            acc = acc + res.results[b * NGROUP + g]["out_t"].astype(np.float32)
        out[b] = acc.T + b_out + vbias
    return out
